# revision 3
# baseline (speedup 1.0000x reference)
"""Trainium2 Bass kernel for nn_CustomAttentionLayer (GQA attention + RoPE + o_proj).

Sharding: 8-way over (batch, query-chunk): core c handles batch c//4, query rows
[(c%4)*512, (c%4)*512+512). Each core computes full K/V for its batch (4x
redundant k/v projection, zero collectives), attention for all 16 heads on its
512 query rows, and the output projection for those rows.

Device layout notes:
- All DRAM inputs are pre-transposed on the host so every DMA is contiguous:
  hT = hidden[b].T with the core's query chunk rotated to columns 0..511,
  wqT/wkT = W.T with rope-permuted columns (even dims then odd dims per head),
  wvT = Wv.T, woT = Wo.T, cos/sin tables transposed and column-rotated the same
  way as hT.
- All matmuls run as float32r (HW rounds inputs to 11-bit mantissa, fp32
  accumulate in PSUM; ~1.6e-4 rel err per matmul, measured on HW).
- Softmax skips the max-subtraction (scores are O(+-5) so exp is safe in fp32
  and the result is mathematically identical); row sums come from a ones-vector
  matmul on the transposed probs; 1/rowsum is broadcast across partitions with
  a K=1 matmul and folded into the attention-output evacuation.
"""

import os
import numpy as np

import concourse.bass as bass
import concourse.mybir as mybir
import concourse.tile as tile
from concourse import bacc
from concourse.bass_utils import run_bass_kernel_spmd

B, S, H = 2, 2048, 2048
NH, NKV, HD = 16, 4, 128
SQ = 512                      # query rows per core
NC = 8                        # cores
KT = H // 128                 # 16 contraction tiles over H
SJ = S // 128                 # 16 key-position tiles
SCALE = 1.0 / float(np.sqrt(HD))

f32 = mybir.dt.float32
f32r = mybir.dt.float32r
FP = mybir.ActivationFunctionType
ALU = mybir.AluOpType


def _body(nc, tc, t):
    """Emit the whole per-core program. `t` maps tensor names -> DRAM APs."""
    hT, wqT, wkT, wvT, woT, cs, outD = (
        t["hT"], t["wqT"], t["wkT"], t["wvT"], t["woT"], t["cs"], t["out"]
    )
    with tc.tile_pool(name="main", bufs=1) as main, tc.tile_pool(
        name="psum", bufs=1, space="PSUM"
    ) as pp:
        cossin = main.tile([128, S], f32, tag="cossin", bufs=1)
        nc.sync.dma_start(cossin[:], cs)
        qts = [main.tile([128, SQ], f32r, tag="qt", bufs=NH, name=f"qt{i}")
               for i in range(NH)]
        kts = [main.tile([128, S], f32r, tag="kt", bufs=NKV, name=f"kt{i}")
               for i in range(NKV)]
        vts = [main.tile([128, NKV * HD], f32r, tag="v", bufs=SJ, name=f"v{i}")
               for i in range(SJ)]

        def rope(dst, ps, c, s, pool):
            # dst[0:64] = ps[0:64]*c - ps[64:128]*s
            # dst[64:128] = ps[0:64]*s + ps[64:128]*c
            w = c.shape[1]
            t1 = pool.tile([64, w], f32, tag="ropet1", bufs=2, name="t1")
            t2 = pool.tile([64, w], f32, tag="ropet2", bufs=2, name="t2")
            nc.vector.tensor_tensor(t1[:], ps[64:128, :], s, op=ALU.mult)
            nc.vector.tensor_tensor(t2[:], ps[0:64, :], c, op=ALU.mult)
            nc.vector.tensor_sub(dst[0:64, :], t2[:], t1[:])
            nc.vector.tensor_tensor(t1[:], ps[0:64, :], s, op=ALU.mult)
            nc.vector.tensor_tensor(t2[:], ps[64:128, :], c, op=ALU.mult)
            nc.vector.tensor_add(dst[64:128, :], t2[:], t1[:])

        # ---------------- projections (streamed over 4 column-quarters) ----
        with tc.tile_pool(name="projtmp", bufs=1) as pt:
            for qt in range(4):
                cols = bass.ts(qt, 512)
                hq = []
                for k in range(KT):
                    hk = pt.tile([128, 512], f32r, tag="hq", bufs=20,
                                 name=f"hq{qt}_{k}")
                    nc.sync.dma_start(hk[:], hT[bass.ts(k, 128), cols])
                    hq.append(hk)

                if qt == 0:
                    # q projection for the core's chunk (= quarter 0)
                    for m in range(NH):
                        ps = pp.tile([128, 512], f32, tag="mm", bufs=4, name="psq")
                        for k in range(KT):
                            wq_k = pt.tile([128, 128], f32r, tag="wq", bufs=4,
                                           name="wqk")
                            nc.sync.dma_start(
                                wq_k[:], wqT[bass.ts(k, 128), bass.ts(m, 128)]
                            )
                            nc.tensor.matmul(
                                ps[:], wq_k[:], hq[k][:],
                                start=(k == 0), stop=(k == KT - 1),
                            )
                        rope(qts[m], ps, cossin[0:64, 0:SQ],
                             cossin[64:128, 0:SQ], pt)

                # k projection: 4 kv heads, this quarter's key positions
                for m in range(NKV):
                    ps = pp.tile([128, 512], f32, tag="mm", bufs=4, name="psk")
                    for k in range(KT):
                        wk_k = pt.tile([128, 128], f32r, tag="wk", bufs=4,
                                       name="wkk")
                        nc.sync.dma_start(
                            wk_k[:], wkT[bass.ts(k, 128), bass.ts(m, 128)]
                        )
                        nc.tensor.matmul(
                            ps[:], wk_k[:], hq[k][:],
                            start=(k == 0), stop=(k == KT - 1),
                        )
                    rope(kts[m][:, cols], ps, cossin[0:64, cols],
                         cossin[64:128, cols], pt)

                # v projection: rows qt*512..+512 of V, all 512 kv dims
                pvs = [pp.tile([128, 512], f32, tag="acc4", bufs=4,
                               name=f"psv{i}") for i in range(4)]
                for k in range(KT):
                    wv_k = pt.tile([128, 512], f32r, tag="wv", bufs=4, name="wvk")
                    nc.sync.dma_start(wv_k[:], wvT[bass.ts(k, 128), :])
                    for sub in range(4):
                        nc.tensor.matmul(
                            pvs[sub][:], hq[k][:, bass.ts(sub, 128)], wv_k[:],
                            start=(k == 0), stop=(k == KT - 1),
                        )
                for sub in range(4):
                    nc.scalar.copy(vts[qt * 4 + sub][:, :], pvs[sub][:])

        # ---------------- attention ----------------------------------------
        with tc.tile_pool(name="attn", bufs=1) as at:
            ones_cf = at.tile([128, 1], f32, tag="ones_cf", bufs=1)
            nc.vector.memset(ones_cf[:], 1.0)
            ones_col = at.tile([128, 1], f32r, tag="ones_col", bufs=1)
            nc.vector.tensor_copy(ones_col[:], ones_cf[:])
            ones_rf = at.tile([1, 128], f32, tag="ones_rf", bufs=1)
            nc.vector.memset(ones_rf[:], 1.0)
            ones_row = at.tile([1, 128], f32r, tag="ones_row", bufs=1)
            nc.vector.tensor_copy(ones_row[:], ones_rf[:])

            onorm = [at.tile([128, SQ], f32r, tag="onorm", bufs=NH,
                             name=f"on{h}") for h in range(NH)]
            for h in range(NH):
                g = h // (NH // NKV)
                pv = pp.tile([128, 512], f32, tag="mm", bufs=4, name="pspv")
                rs = pp.tile([1, 512], f32, tag="mm", bufs=4, name="psrs")
                for j in range(SJ):
                    sc = pp.tile([128, 512], f32, tag="mm", bufs=4, name="pssc")
                    nc.tensor.matmul(
                        sc[:], kts[g][:, bass.ts(j, 128)], qts[h][:],
                        start=True, stop=True,
                    )
                    ex = at.tile([128, 512], f32r, tag="expt", bufs=20, name="ex")
                    nc.scalar.activation(ex[:], sc[:], FP.Exp, scale=SCALE)
                    nc.tensor.matmul(
                        rs[:], ones_col[:], ex[:],
                        start=(j == 0), stop=(j == SJ - 1),
                    )
                    nc.tensor.matmul(
                        pv[:], vts[j][:, bass.ts(g, 128)], ex[:],
                        start=(j == 0), stop=(j == SJ - 1),
                    )
                recip = at.tile([1, 512], f32r, tag="recip", bufs=2, name="rc")
                with nc.allow_low_precision(reason="1/rowsum feeds an f32r matmul"):
                    nc.vector.reciprocal(recip[:], rs[:])
                bc = pp.tile([128, 512], f32, tag="mm", bufs=4, name="psbc")
                nc.tensor.matmul(bc[:], ones_row[:], recip[:],
                                 start=True, stop=True)
                bcs = at.tile([128, 512], f32, tag="bcs", bufs=2, name="bcs")
                nc.scalar.copy(bcs[:], bc[:])
                nc.vector.tensor_tensor(onorm[h][:, :], pv[:], bcs[:],
                                        op=ALU.mult)

            # ---------------- output projection ----------------------------
            with tc.tile_pool(name="oproj", bufs=1) as ot:
                for n in range(4):
                    psos = [pp.tile([128, 512], f32, tag="acc4", bufs=4,
                                    name=f"pso{i}") for i in range(4)]
                    for h in range(NH):
                        wo_t = ot.tile([128, 512], f32r, tag="wo", bufs=4,
                                       name="wot")
                        nc.sync.dma_start(
                            wo_t[:], woT[bass.ts(h, 128), bass.ts(n, 512)]
                        )
                        for sqt in range(4):
                            nc.tensor.matmul(
                                psos[sqt][:],
                                onorm[h][:, bass.ts(sqt, 128)], wo_t[:],
                                start=(h == 0), stop=(h == NH - 1),
                            )
                    for sqt in range(4):
                        o_s = ot.tile([128, 512], f32, tag="osb", bufs=3,
                                      name="osb")
                        nc.scalar.copy(o_s[:], psos[sqt][:])
                        nc.sync.dma_start(
                            outD[bass.ts(sqt, 128), bass.ts(n, 512)], o_s[:]
                        )


def build(reps=1):
    nc = bacc.Bacc("TRN2", target_bir_lowering=False, debug=False,
                   num_devices=NC)
    t = {
        "hT": nc.dram_tensor("hT", [H, S], f32r, kind="ExternalInput").ap(),
        "wqT": nc.dram_tensor("wqT", [H, H], f32r, kind="ExternalInput").ap(),
        "wkT": nc.dram_tensor("wkT", [H, NKV * HD], f32r,
                              kind="ExternalInput").ap(),
        "wvT": nc.dram_tensor("wvT", [H, NKV * HD], f32r,
                              kind="ExternalInput").ap(),
        "woT": nc.dram_tensor("woT", [H, H], f32r, kind="ExternalInput").ap(),
        "cs": nc.dram_tensor("cs", [128, S], f32, kind="ExternalInput").ap(),
        "out": nc.dram_tensor("out", [SQ, H], f32, kind="ExternalOutput").ap(),
    }
    with tile.TileContext(nc) as tc:
        for _ in range(reps):
            _body(nc, tc, t)
    nc.compile()
    return nc


_ROPE_PERM = np.concatenate(
    [h * HD + np.r_[np.arange(0, HD, 2), np.arange(1, HD, 2)]
     for h in range(NH)]
)
_ROPE_PERM_KV = _ROPE_PERM[: NKV * HD]


def prep_inputs(hidden_states, freqs_cos, freqs_sin, Wq, Wk, Wv, Wo):
    """Host-side layout prep -> list of 8 per-core input maps."""
    wqT = np.ascontiguousarray(Wq.T[:, _ROPE_PERM])
    wkT = np.ascontiguousarray(Wk.T[:, _ROPE_PERM_KV])
    wvT = np.ascontiguousarray(Wv.T)
    woT = np.ascontiguousarray(Wo.T)
    cosT = freqs_cos.T  # [64, S]
    sinT = freqs_sin.T
    in_maps = []
    for c in range(NC):
        b, chunk = divmod(c, 4)
        sq0 = chunk * SQ
        perm = np.r_[sq0: sq0 + SQ, 0:sq0, sq0 + SQ: S]
        hTc = np.ascontiguousarray(hidden_states[b].T[:, perm])
        csc = np.ascontiguousarray(
            np.concatenate([cosT[:, perm], sinT[:, perm]], 0)
        )
        in_maps.append(
            {"hT": hTc, "wqT": wqT, "wkT": wkT, "wvT": wvT, "woT": woT,
             "cs": csc}
        )
    return in_maps


_CACHE = {}


def _get_nc(reps=1):
    if reps not in _CACHE:
        _CACHE[reps] = build(reps)
    return _CACHE[reps]


def kernel(hidden_states, freqs_cos, freqs_sin, Wq, Wk, Wv, Wo):
    in_maps = prep_inputs(
        np.asarray(hidden_states, np.float32),
        np.asarray(freqs_cos, np.float32),
        np.asarray(freqs_sin, np.float32),
        np.asarray(Wq, np.float32),
        np.asarray(Wk, np.float32),
        np.asarray(Wv, np.float32),
        np.asarray(Wo, np.float32),
    )
    nc = _get_nc(int(os.environ.get("KERNEL_REPS", "1")))
    res = run_bass_kernel_spmd(nc, in_maps, core_ids=list(range(NC)))
    out = np.empty((B, S, H), np.float32)
    for c in range(NC):
        b, chunk = divmod(c, 4)
        out[b, chunk * SQ: (chunk + 1) * SQ, :] = res.results[c]["out"]
    return out


# revision 19
# speedup vs baseline: 722.8849x; 722.8849x over previous
"""Trainium2 Bass kernel for nn_CustomAttentionLayer (GQA attention + RoPE + o_proj).

Sharding: 8-way over (batch, query-chunk): core c handles batch c//4, query rows
[(c%4)*512, (c%4)*512+512). Each core computes full K/V for its batch (4x
redundant k/v projection, zero collectives), attention for all 16 heads on its
512 query rows, and the output projection for those rows.

Device layout notes:
- All DRAM inputs are pre-transposed on the host so every DMA is contiguous:
  hT = hidden[b].T with the core's query chunk rotated to columns 0..511,
  wqT/wkT = W.T with rope-permuted columns (even dims then odd dims per head),
  wvT = Wv.T, woT = Wo.T, cos/sin tables transposed and column-rotated the same
  way as hT.
- All matmuls run as float32r (HW rounds inputs to 11-bit mantissa, fp32
  accumulate in PSUM; ~1.6e-4 rel err per matmul, measured on HW).
- Softmax skips the max-subtraction (scores are O(+-5) so exp is safe in fp32
  and the result is mathematically identical); row sums come from a ones-vector
  matmul on the transposed probs; 1/rowsum is broadcast across partitions with
  a K=1 matmul and folded into the attention-output evacuation.
"""

import os
import numpy as np

import concourse.bass as bass
import concourse.mybir as mybir
import concourse.tile as tile
from concourse import bacc
from concourse.bass_utils import run_bass_kernel_spmd

B, S, H = 2, 2048, 2048
NH, NKV, HD = 16, 4, 128
SQ = 512                      # query rows per core
NC = 8                        # cores
KT = H // 128                 # 16 contraction tiles over H
SJ = S // 128                 # 16 key-position tiles
SCALE = 1.0 / float(np.sqrt(HD))

f32 = mybir.dt.float32
f32r = mybir.dt.float32r
FP = mybir.ActivationFunctionType
ALU = mybir.AluOpType


def _body(nc, tc, t):
    """Emit the whole per-core program. `t` maps tensor names -> DRAM APs."""
    hT, wqT, wkT, wvT, woT, ccD, ssD, outD = (
        t["hT"], t["wqT"], t["wkT"], t["wvT"], t["woT"], t["cc"], t["ss"],
        t["out"]
    )
    with tc.tile_pool(name="main", bufs=1) as main, tc.tile_pool(
        name="psum", bufs=1, space="PSUM"
    ) as pp:
        # cc = [cos; cos], ss = [sin; sin] stacked on the partition dim
        # (DMA emission deferred so the first wq/hq loads go out first)
        cc = main.tile([128, S], f32, tag="cc", bufs=1)
        ss = main.tile([128, S], f32, tag="ss", bufs=1)
        qts = [main.tile([128, SQ], f32r, tag="qt", bufs=NH, name=f"qt{i}")
               for i in range(NH)]
        kts = [main.tile([128, S], f32r, tag="kt", bufs=NKV, name=f"kt{i}")
               for i in range(NKV)]
        vts = [main.tile([128, NKV * HD], f32r, tag="v", bufs=SJ, name=f"v{i}")
               for i in range(SJ)]

        def rope(dst, ps, cols, pool):
            # With x = [xr; xi] on partition halves and cc = [c; c], ss = [s; s]:
            #   A = x*cc = (xr*c | xi*c),  B = x*ss = (xr*s | xi*s)
            #   dst[0:64]   = xr*c - xi*s = A[0:64]  - B[64:128]
            #   dst[64:128] = xr*s + xi*c = B[0:64]  + A[64:128]
            w = dst.shape[-1]
            t1 = pool.tile([64, w], f32, tag="ropeA", bufs=2, name="t1")
            t2 = pool.tile([64, w], f32, tag="ropeB", bufs=1, name="t2")
            nc.vector.tensor_tensor(t1[:], ps[64:128, :], ss[64:128, cols],
                                    op=ALU.mult)
            nc.vector.tensor_tensor(t2[:], ps[0:64, :], cc[0:64, cols],
                                    op=ALU.mult)
            nc.vector.tensor_sub(dst[0:64, :], t2[:], t1[:])
            nc.vector.tensor_tensor(t1[:], ps[0:64, :], ss[0:64, cols],
                                    op=ALU.mult)
            nc.vector.tensor_tensor(t2[:], ps[64:128, :], cc[64:128, cols],
                                    op=ALU.mult)
            nc.vector.tensor_add(dst[64:128, :], t2[:], t1[:])

        # ---------------- projections (streamed over 4 column-quarters) ----
        # Weight DRAM views packed so one DMA loads all 16 contraction tiles
        # of a 128-wide output block: [H, n] -> [p, k, n-slice].
        wqT_p = wqT.rearrange("(k p) n -> p k n", p=128)
        wkT_p = wkT.rearrange("(k p) n -> p k n", p=128)
        wvT_p = wvT.rearrange("(k p) n -> p k n", p=128)
        with tc.tile_pool(name="projtmp", bufs=1) as pt:
            wkr = []

            def load_wk():
                for m in range(NKV):
                    wkm = pt.tile([128, KT * 128], f32r, tag="wkr", bufs=NKV,
                                  name=f"wkr{m}")
                    nc.sync.dma_start(
                        wkm[:].rearrange("p (k j) -> p k j", j=128),
                        wkT_p[:, :, bass.ts(m, 128)],
                    )
                    wkr.append(wkm)

            wqm_tiles = {}

            def prefetch_wq(m):
                if m < NH:
                    wqm = pt.tile([128, KT * 128], f32r, tag="wqm", bufs=2,
                                  name="wqm")
                    nc.sync.dma_start(
                        wqm[:].rearrange("p (k j) -> p k j", j=128),
                        wqT_p[:, :, bass.ts(m, 128)],
                    )
                    wqm_tiles[m] = wqm

            prefetch_wq(0)

            for qt in range(4):
                cols = bass.ts(qt, 512)
                hq = []
                for k in range(KT):
                    hk = pt.tile([128, 512], f32r, tag="hq", bufs=16,
                                 name=f"hq{qt}_{k}")
                    nc.sync.dma_start(hk[:], hT[bass.ts(k, 128), cols])
                    hq.append(hk)

                # rope tables for this quarter's columns only (keeps the
                # startup DMA queue clear for the first weight blocks)
                nc.sync.dma_start(cc[:, cols], ccD[:, cols])
                nc.sync.dma_start(ss[:, cols], ssD[:, cols])

                if qt == 0:
                    # q projection for the core's chunk (= quarter 0)
                    for m in range(NH):
                        wqm = wqm_tiles.pop(m)
                        prefetch_wq(m + 1)
                        ps = pp.tile([128, 512], f32, tag="mm", bufs=4, name="psq")
                        for k in range(KT):
                            nc.tensor.matmul(
                                ps[:], wqm[:, bass.ts(k, 128)], hq[k][:],
                                start=(k == 0), stop=(k == KT - 1),
                            )
                        rope(qts[m], ps, slice(0, SQ), pt)
                    load_wk()

                # k projection: 4 kv heads, this quarter's key positions
                for m in range(NKV):
                    ps = pp.tile([128, 512], f32, tag="mm", bufs=4, name="psk")
                    for k in range(KT):
                        nc.tensor.matmul(
                            ps[:], wkr[m][:, bass.ts(k, 128)], hq[k][:],
                            start=(k == 0), stop=(k == KT - 1),
                        )
                    rope(kts[m][:, cols], ps, cols, pt)

                # v projection for this quarter's rows; Wv streamed per k-tile
                pvs = [pp.tile([128, 512], f32, tag="acc4", bufs=4,
                               name=f"psv{i}") for i in range(4)]
                for k in range(KT):
                    wv_k = pt.tile([128, 512], f32r, tag="wv", bufs=4,
                                   name="wvk")
                    nc.sync.dma_start(wv_k[:], wvT[bass.ts(k, 128), :])
                    for sub in range(4):
                        nc.tensor.matmul(
                            pvs[sub][:], hq[k][:, bass.ts(sub, 128)], wv_k[:],
                            start=(k == 0), stop=(k == KT - 1),
                        )
                for sub in range(4):
                    nc.scalar.copy(vts[qt * 4 + sub][:, :], pvs[sub][:])

        # ---------------- attention ----------------------------------------
        with tc.tile_pool(name="attn", bufs=1) as at:
            ones_f = at.tile([128, 128], f32, tag="ones_f", bufs=1)
            nc.vector.memset(ones_f[:], 1.0)
            ones_sq = at.tile([128, 128], f32r, tag="ones_sq", bufs=1)
            nc.vector.tensor_copy(ones_sq[:], ones_f[:])

            onorm = [at.tile([128, SQ], f32r, tag="onorm", bufs=NH,
                             name=f"on{h}") for h in range(NH)]
            for h in range(NH):
                g = h // (NH // NKV)
                pv = pp.tile([128, 512], f32, tag="acc4", bufs=4, name="pspv")
                # rowsum broadcast to all 128 partitions: ones[128,128].T @ ex
                rsb = pp.tile([128, 512], f32, tag="mm", bufs=4, name="psrs")
                for j in range(SJ):
                    sc = pp.tile([128, 512], f32, tag="mm", bufs=4, name="pssc")
                    nc.tensor.matmul(
                        sc[:], kts[g][:, bass.ts(j, 128)], qts[h][:],
                        start=True, stop=True,
                    )
                    ex = at.tile([128, 512], f32r, tag="expt", bufs=20, name="ex")
                    nc.scalar.activation(ex[:], sc[:], FP.Exp, scale=SCALE)
                    nc.tensor.matmul(
                        rsb[:], ones_sq[:], ex[:],
                        start=(j == 0), stop=(j == SJ - 1),
                    )
                    nc.tensor.matmul(
                        pv[:], vts[j][:, bass.ts(g, 128)], ex[:],
                        start=(j == 0), stop=(j == SJ - 1),
                    )
                recipb = at.tile([128, 512], f32, tag="recipb", bufs=2,
                                 name="rc")
                with nc.allow_low_precision(reason="1/rowsum feeds f32r mul"):
                    nc.vector.reciprocal(recipb[:], rsb[:])
                nc.vector.tensor_tensor(onorm[h][:, :], pv[:], recipb[:],
                                        op=ALU.mult)

            # ---------------- output projection ----------------------------
            with tc.tile_pool(name="oproj", bufs=1) as ot:
                for n in range(4):
                    psos = [pp.tile([128, 512], f32, tag="acc4", bufs=4,
                                    name=f"pso{i}") for i in range(4)]
                    for h in range(NH):
                        wo_t = ot.tile([128, 512], f32r, tag="wo", bufs=6,
                                       name="wot")
                        nc.sync.dma_start(
                            wo_t[:], woT[bass.ts(h, 128), bass.ts(n, 512)]
                        )
                        for sqt in range(4):
                            nc.tensor.matmul(
                                psos[sqt][:],
                                onorm[h][:, bass.ts(sqt, 128)], wo_t[:],
                                start=(h == 0), stop=(h == NH - 1),
                            )
                    for sqt in range(4):
                        o_s = ot.tile([128, 512], f32, tag="osb", bufs=2,
                                      name="osb")
                        nc.scalar.copy(o_s[:], psos[sqt][:])
                        nc.sync.dma_start(
                            outD[bass.ts(sqt, 128), bass.ts(n, 512)], o_s[:]
                        )


def build(reps=1):
    nc = bacc.Bacc("TRN2", target_bir_lowering=False, debug=False,
                   num_devices=NC)
    t = {
        "hT": nc.dram_tensor("hT", [H, S], f32r, kind="ExternalInput").ap(),
        "wqT": nc.dram_tensor("wqT", [H, H], f32r, kind="ExternalInput").ap(),
        "wkT": nc.dram_tensor("wkT", [H, NKV * HD], f32r,
                              kind="ExternalInput").ap(),
        "wvT": nc.dram_tensor("wvT", [H, NKV * HD], f32r,
                              kind="ExternalInput").ap(),
        "woT": nc.dram_tensor("woT", [H, H], f32r, kind="ExternalInput").ap(),
        "cc": nc.dram_tensor("cc", [128, S], f32, kind="ExternalInput").ap(),
        "ss": nc.dram_tensor("ss", [128, S], f32, kind="ExternalInput").ap(),
        "out": nc.dram_tensor("out", [SQ, H], f32, kind="ExternalOutput").ap(),
    }
    with tile.TileContext(nc) as tc:
        for _ in range(reps):
            _body(nc, tc, t)
    nc.compile()
    return nc


_ROPE_PERM = np.concatenate(
    [h * HD + np.r_[np.arange(0, HD, 2), np.arange(1, HD, 2)]
     for h in range(NH)]
)
_ROPE_PERM_KV = _ROPE_PERM[: NKV * HD]


def prep_inputs(hidden_states, freqs_cos, freqs_sin, Wq, Wk, Wv, Wo):
    """Host-side layout prep -> list of 8 per-core input maps."""
    wqT = np.ascontiguousarray(Wq.T[:, _ROPE_PERM])
    wkT = np.ascontiguousarray(Wk.T[:, _ROPE_PERM_KV])
    wvT = np.ascontiguousarray(Wv.T)
    woT = np.ascontiguousarray(Wo.T)
    cosT = freqs_cos.T  # [64, S]
    sinT = freqs_sin.T
    in_maps = []
    for c in range(NC):
        b, chunk = divmod(c, 4)
        sq0 = chunk * SQ
        perm = np.r_[sq0: sq0 + SQ, 0:sq0, sq0 + SQ: S]
        hTc = np.ascontiguousarray(hidden_states[b].T[:, perm])
        ccc = np.ascontiguousarray(np.concatenate([cosT, cosT], 0)[:, perm])
        ssc = np.ascontiguousarray(np.concatenate([sinT, sinT], 0)[:, perm])
        in_maps.append(
            {"hT": hTc, "wqT": wqT, "wkT": wkT, "wvT": wvT, "woT": woT,
             "cc": ccc, "ss": ssc}
        )
    return in_maps


_CACHE = {}


def _get_nc(reps=1):
    if reps not in _CACHE:
        _CACHE[reps] = build(reps)
    return _CACHE[reps]


def kernel(hidden_states, freqs_cos, freqs_sin, Wq, Wk, Wv, Wo):
    in_maps = prep_inputs(
        np.asarray(hidden_states, np.float32),
        np.asarray(freqs_cos, np.float32),
        np.asarray(freqs_sin, np.float32),
        np.asarray(Wq, np.float32),
        np.asarray(Wk, np.float32),
        np.asarray(Wv, np.float32),
        np.asarray(Wo, np.float32),
    )
    nc = _get_nc(int(os.environ.get("KERNEL_REPS", "1")))
    res = run_bass_kernel_spmd(nc, in_maps, core_ids=list(range(NC)))
    out = np.empty((B, S, H), np.float32)
    for c in range(NC):
        b, chunk = divmod(c, 4)
        out[b, chunk * SQ: (chunk + 1) * SQ, :] = res.results[c]["out"]
    return out


# revision 20
# speedup vs baseline: 767.4724x; 1.0617x over previous
"""Trainium2 Bass kernel for nn_CustomAttentionLayer (GQA attention + RoPE + o_proj).

Sharding: 8-way over (batch, query-chunk): core c handles batch c//4, query rows
[(c%4)*512, (c%4)*512+512). Each core computes full K/V for its batch (4x
redundant k/v projection, zero collectives), attention for all 16 heads on its
512 query rows, and the output projection for those rows.

Device layout notes:
- All DRAM inputs are pre-transposed on the host so every DMA is contiguous:
  hT = hidden[b].T with the core's query chunk rotated to columns 0..511,
  wqT/wkT = W.T with rope-permuted columns (even dims then odd dims per head),
  wvT = Wv.T, woT = Wo.T, cos/sin tables transposed and column-rotated the same
  way as hT.
- All matmuls run as float32r (HW rounds inputs to 11-bit mantissa, fp32
  accumulate in PSUM; ~1.6e-4 rel err per matmul, measured on HW).
- Softmax skips the max-subtraction (scores are O(+-5) so exp is safe in fp32
  and the result is mathematically identical); row sums come from a ones-vector
  matmul on the transposed probs; 1/rowsum is broadcast across partitions with
  a K=1 matmul and folded into the attention-output evacuation.
"""

import os
import numpy as np

import concourse.bass as bass
import concourse.mybir as mybir
import concourse.tile as tile
from concourse import bacc
from concourse.bass_utils import run_bass_kernel_spmd

B, S, H = 2, 2048, 2048
NH, NKV, HD = 16, 4, 128
SQ = 512                      # query rows per core
NC = 8                        # cores
KT = H // 128                 # 16 contraction tiles over H
SJ = S // 128                 # 16 key-position tiles
SCALE = 1.0 / float(np.sqrt(HD))

f32 = mybir.dt.float32
f32r = mybir.dt.float32r
FP = mybir.ActivationFunctionType
ALU = mybir.AluOpType


def _body(nc, tc, t):
    """Emit the whole per-core program. `t` maps tensor names -> DRAM APs."""
    hT, wqT, wkT, wvT, woT, ccD, ssD, outD = (
        t["hT"], t["wqT"], t["wkT"], t["wvT"], t["woT"], t["cc"], t["ss"],
        t["out"]
    )
    with tc.tile_pool(name="main", bufs=1) as main, tc.tile_pool(
        name="psum", bufs=1, space="PSUM"
    ) as pp:
        # cc = [cos; cos], ss = [sin; sin] stacked on the partition dim
        # (DMA emission deferred so the first wq/hq loads go out first)
        cc = main.tile([128, S], f32, tag="cc", bufs=1)
        ss = main.tile([128, S], f32, tag="ss", bufs=1)
        qts = [main.tile([128, SQ], f32r, tag="qt", bufs=NH, name=f"qt{i}")
               for i in range(NH)]
        kts = [main.tile([128, S], f32r, tag="kt", bufs=NKV, name=f"kt{i}")
               for i in range(NKV)]
        vts = [main.tile([128, NKV * HD], f32r, tag="v", bufs=SJ, name=f"v{i}")
               for i in range(SJ)]

        def rope(dst, ps, cols, pool):
            # With x = [xr; xi] on partition halves and cc = [c; c], ss = [s; s]:
            #   A = x*cc = (xr*c | xi*c),  B = x*ss = (xr*s | xi*s)
            #   dst[0:64]   = xr*c - xi*s = A[0:64]  - B[64:128]
            #   dst[64:128] = xr*s + xi*c = B[0:64]  + A[64:128]
            w = dst.shape[-1]
            t1 = pool.tile([64, w], f32, tag="ropeA", bufs=2, name="t1")
            t2 = pool.tile([64, w], f32, tag="ropeB", bufs=1, name="t2")
            nc.vector.tensor_tensor(t1[:], ps[64:128, :], ss[64:128, cols],
                                    op=ALU.mult)
            nc.vector.tensor_tensor(t2[:], ps[0:64, :], cc[0:64, cols],
                                    op=ALU.mult)
            nc.vector.tensor_sub(dst[0:64, :], t2[:], t1[:])
            nc.vector.tensor_tensor(t1[:], ps[0:64, :], ss[0:64, cols],
                                    op=ALU.mult)
            nc.vector.tensor_tensor(t2[:], ps[64:128, :], cc[64:128, cols],
                                    op=ALU.mult)
            nc.vector.tensor_add(dst[64:128, :], t2[:], t1[:])

        # ---------------- projections (streamed over 4 column-quarters) ----
        # Weight DRAM views packed so one DMA loads all 16 contraction tiles
        # of a 128-wide output block: [H, n] -> [p, k, n-slice].
        wqT_p = wqT.rearrange("(k p) n -> p k n", p=128)
        wkT_p = wkT.rearrange("(k p) n -> p k n", p=128)
        wvT_p = wvT.rearrange("(k p) n -> p k n", p=128)
        with tc.tile_pool(name="projtmp", bufs=1) as pt:
            wkr = []

            def load_wk():
                for m in range(NKV):
                    wkm = pt.tile([128, KT * 128], f32r, tag="wkr", bufs=NKV,
                                  name=f"wkr{m}")
                    nc.sync.dma_start(
                        wkm[:].rearrange("p (k j) -> p k j", j=128),
                        wkT_p[:, :, bass.ts(m, 128)],
                    )
                    wkr.append(wkm)

            wqm_tiles = {}

            def prefetch_wq(m):
                if m < NH:
                    wqm = pt.tile([128, KT * 128], f32r, tag="wqm", bufs=2,
                                  name="wqm")
                    nc.sync.dma_start(
                        wqm[:].rearrange("p (k j) -> p k j", j=128),
                        wqT_p[:, :, bass.ts(m, 128)],
                    )
                    wqm_tiles[m] = wqm

            prefetch_wq(0)

            for qt in range(4):
                cols = bass.ts(qt, 512)
                hq = []
                for k in range(KT):
                    hk = pt.tile([128, 512], f32r, tag="hq", bufs=16,
                                 name=f"hq{qt}_{k}")
                    nc.sync.dma_start(hk[:], hT[bass.ts(k, 128), cols])
                    hq.append(hk)

                # rope tables for this quarter's columns only (keeps the
                # startup DMA queue clear for the first weight blocks)
                if qt == 0:
                    prefetch_wq(1)
                nc.sync.dma_start(cc[:, cols], ccD[:, cols])
                nc.sync.dma_start(ss[:, cols], ssD[:, cols])

                if qt == 0:
                    # q projection for the core's chunk (= quarter 0)
                    for m in range(NH):
                        wqm = wqm_tiles.pop(m)
                        prefetch_wq(m + 2)
                        ps = pp.tile([128, 512], f32, tag="mm", bufs=4, name="psq")
                        for k in range(KT):
                            nc.tensor.matmul(
                                ps[:], wqm[:, bass.ts(k, 128)], hq[k][:],
                                start=(k == 0), stop=(k == KT - 1),
                            )
                        rope(qts[m], ps, slice(0, SQ), pt)
                    load_wk()

                # k projection: 4 kv heads, this quarter's key positions
                for m in range(NKV):
                    ps = pp.tile([128, 512], f32, tag="mm", bufs=4, name="psk")
                    for k in range(KT):
                        nc.tensor.matmul(
                            ps[:], wkr[m][:, bass.ts(k, 128)], hq[k][:],
                            start=(k == 0), stop=(k == KT - 1),
                        )
                    rope(kts[m][:, cols], ps, cols, pt)

                # v projection for this quarter's rows; Wv streamed per k-tile
                pvs = [pp.tile([128, 512], f32, tag="acc4", bufs=4,
                               name=f"psv{i}") for i in range(4)]
                for k in range(KT):
                    wv_k = pt.tile([128, 512], f32r, tag="wv", bufs=4,
                                   name="wvk")
                    nc.sync.dma_start(wv_k[:], wvT[bass.ts(k, 128), :])
                    for sub in range(4):
                        nc.tensor.matmul(
                            pvs[sub][:], hq[k][:, bass.ts(sub, 128)], wv_k[:],
                            start=(k == 0), stop=(k == KT - 1),
                        )
                for sub in range(4):
                    nc.scalar.copy(vts[qt * 4 + sub][:, :], pvs[sub][:])

        # ---------------- attention ----------------------------------------
        with tc.tile_pool(name="attn", bufs=1) as at:
            ones_f = at.tile([128, 128], f32, tag="ones_f", bufs=1)
            nc.vector.memset(ones_f[:], 1.0)
            ones_sq = at.tile([128, 128], f32r, tag="ones_sq", bufs=1)
            nc.vector.tensor_copy(ones_sq[:], ones_f[:])

            onorm = [at.tile([128, SQ], f32r, tag="onorm", bufs=NH,
                             name=f"on{h}") for h in range(NH)]
            for h in range(NH):
                g = h // (NH // NKV)
                pv = pp.tile([128, 512], f32, tag="acc4", bufs=4, name="pspv")
                # rowsum broadcast to all 128 partitions: ones[128,128].T @ ex
                rsb = pp.tile([128, 512], f32, tag="mm", bufs=4, name="psrs")
                for j in range(SJ):
                    sc = pp.tile([128, 512], f32, tag="mm", bufs=4, name="pssc")
                    nc.tensor.matmul(
                        sc[:], kts[g][:, bass.ts(j, 128)], qts[h][:],
                        start=True, stop=True,
                    )
                    ex = at.tile([128, 512], f32r, tag="expt", bufs=20, name="ex")
                    nc.scalar.activation(ex[:], sc[:], FP.Exp, scale=SCALE)
                    nc.tensor.matmul(
                        rsb[:], ones_sq[:], ex[:],
                        start=(j == 0), stop=(j == SJ - 1),
                    )
                    nc.tensor.matmul(
                        pv[:], vts[j][:, bass.ts(g, 128)], ex[:],
                        start=(j == 0), stop=(j == SJ - 1),
                    )
                recipb = at.tile([128, 512], f32, tag="recipb", bufs=2,
                                 name="rc")
                with nc.allow_low_precision(reason="1/rowsum feeds f32r mul"):
                    nc.vector.reciprocal(recipb[:], rsb[:])
                nc.vector.tensor_tensor(onorm[h][:, :], pv[:], recipb[:],
                                        op=ALU.mult)

            # ---------------- output projection ----------------------------
            with tc.tile_pool(name="oproj", bufs=1) as ot:
                for n in range(4):
                    psos = [pp.tile([128, 512], f32, tag="acc4", bufs=4,
                                    name=f"pso{i}") for i in range(4)]
                    for h in range(NH):
                        wo_t = ot.tile([128, 512], f32r, tag="wo", bufs=7,
                                       name="wot")
                        nc.sync.dma_start(
                            wo_t[:], woT[bass.ts(h, 128), bass.ts(n, 512)]
                        )
                        for sqt in range(4):
                            nc.tensor.matmul(
                                psos[sqt][:],
                                onorm[h][:, bass.ts(sqt, 128)], wo_t[:],
                                start=(h == 0), stop=(h == NH - 1),
                            )
                    for sqt in range(4):
                        o_s = ot.tile([128, 512], f32, tag="osb", bufs=2,
                                      name="osb")
                        nc.scalar.copy(o_s[:], psos[sqt][:])
                        nc.sync.dma_start(
                            outD[bass.ts(sqt, 128), bass.ts(n, 512)], o_s[:]
                        )


def build(reps=1):
    nc = bacc.Bacc("TRN2", target_bir_lowering=False, debug=False,
                   num_devices=NC)
    t = {
        "hT": nc.dram_tensor("hT", [H, S], f32r, kind="ExternalInput").ap(),
        "wqT": nc.dram_tensor("wqT", [H, H], f32r, kind="ExternalInput").ap(),
        "wkT": nc.dram_tensor("wkT", [H, NKV * HD], f32r,
                              kind="ExternalInput").ap(),
        "wvT": nc.dram_tensor("wvT", [H, NKV * HD], f32r,
                              kind="ExternalInput").ap(),
        "woT": nc.dram_tensor("woT", [H, H], f32r, kind="ExternalInput").ap(),
        "cc": nc.dram_tensor("cc", [128, S], f32, kind="ExternalInput").ap(),
        "ss": nc.dram_tensor("ss", [128, S], f32, kind="ExternalInput").ap(),
        "out": nc.dram_tensor("out", [SQ, H], f32, kind="ExternalOutput").ap(),
    }
    with tile.TileContext(nc) as tc:
        for _ in range(reps):
            _body(nc, tc, t)
    nc.compile()
    return nc


_ROPE_PERM = np.concatenate(
    [h * HD + np.r_[np.arange(0, HD, 2), np.arange(1, HD, 2)]
     for h in range(NH)]
)
_ROPE_PERM_KV = _ROPE_PERM[: NKV * HD]


def prep_inputs(hidden_states, freqs_cos, freqs_sin, Wq, Wk, Wv, Wo):
    """Host-side layout prep -> list of 8 per-core input maps."""
    wqT = np.ascontiguousarray(Wq.T[:, _ROPE_PERM])
    wkT = np.ascontiguousarray(Wk.T[:, _ROPE_PERM_KV])
    wvT = np.ascontiguousarray(Wv.T)
    woT = np.ascontiguousarray(Wo.T)
    cosT = freqs_cos.T  # [64, S]
    sinT = freqs_sin.T
    in_maps = []
    for c in range(NC):
        b, chunk = divmod(c, 4)
        sq0 = chunk * SQ
        perm = np.r_[sq0: sq0 + SQ, 0:sq0, sq0 + SQ: S]
        hTc = np.ascontiguousarray(hidden_states[b].T[:, perm])
        ccc = np.ascontiguousarray(np.concatenate([cosT, cosT], 0)[:, perm])
        ssc = np.ascontiguousarray(np.concatenate([sinT, sinT], 0)[:, perm])
        in_maps.append(
            {"hT": hTc, "wqT": wqT, "wkT": wkT, "wvT": wvT, "woT": woT,
             "cc": ccc, "ss": ssc}
        )
    return in_maps


_CACHE = {}


def _get_nc(reps=1):
    if reps not in _CACHE:
        _CACHE[reps] = build(reps)
    return _CACHE[reps]


def kernel(hidden_states, freqs_cos, freqs_sin, Wq, Wk, Wv, Wo):
    in_maps = prep_inputs(
        np.asarray(hidden_states, np.float32),
        np.asarray(freqs_cos, np.float32),
        np.asarray(freqs_sin, np.float32),
        np.asarray(Wq, np.float32),
        np.asarray(Wk, np.float32),
        np.asarray(Wv, np.float32),
        np.asarray(Wo, np.float32),
    )
    nc = _get_nc(int(os.environ.get("KERNEL_REPS", "1")))
    res = run_bass_kernel_spmd(nc, in_maps, core_ids=list(range(NC)))
    out = np.empty((B, S, H), np.float32)
    for c in range(NC):
        b, chunk = divmod(c, 4)
        out[b, chunk * SQ: (chunk + 1) * SQ, :] = res.results[c]["out"]
    return out
